# revision 22
# baseline (speedup 1.0000x reference)
"""Trainium2 Bass kernel for nn_MHA_9603546874182.

Causal MHA: qkv proj + rope(32) + causal attention + out proj.
B=4, T=1024, C=2048, H=32, hd=64.

Sharding: 8-way tensor parallel over heads (4 heads / core).
Each core computes qkv for its 4 heads (column-parallel), rope,
causal attention, and a row-parallel partial of the output
projection. Host sums the 8 bf16 partials (+ bias, incl. the v-bias
contribution folded through out_w).

All matmuls run in bf16 (1 cycle/row on the PE at 2.4 GHz vs the
2-4x slower fp32r path measured on HW). Work is spread across
engines: DVE does rope/bias/normalize, ACT does exp + the q/k
transpose-psum evacuations, Pool (gpsimd) does mask adds + psum->
sbuf copies for v and the output, PE does matmuls + transposes.
"""

import numpy as np

B, T, C, H = 4, 1024, 2048, 32
HD = C // H          # 64
NCORES = 8
HPC = H // NCORES    # 4 heads per core
SC = HPC * HD        # 256 shard channels
NTOK = B * T         # 4096
KT16 = C // 128      # 16 k tiles
MT = NTOK // 128     # 32 token tiles
MPB = T // 128       # 8 token tiles per batch
ROT = 32
NEG = -1.0e9

_CACHE = {}


def _build_nc():
    import concourse.bass as bass
    import concourse.mybir as mybir
    import concourse.tile as tile
    from concourse import bacc
    from concourse.masks import make_identity

    f32 = mybir.dt.float32
    bf16 = mybir.dt.bfloat16

    nc = bacc.Bacc("TRN2")

    xt_d = nc.dram_tensor("xt", [128, MT, KT16 * 128], bf16, kind="ExternalInput")
    wq_d = nc.dram_tensor("wq", [128, KT16, 3 * SC], bf16, kind="ExternalInput")
    br_d = nc.dram_tensor("br", [128, 2 * SC], f32, kind="ExternalInput")
    c1_d = nc.dram_tensor("c1", [128, MPB, SC], f32, kind="ExternalInput")
    c2_d = nc.dram_tensor("c2", [128, MPB, SC], f32, kind="ExternalInput")
    mk_d = nc.dram_tensor("mk", [128, 1024], bf16, kind="ExternalInput")
    w2_d = nc.dram_tensor("w2", [128, 2, C], bf16, kind="ExternalInput")
    out_d = nc.dram_tensor("out", [MT, 128, C], bf16, kind="ExternalOutput")

    with tile.TileContext(nc) as tc:
        with (
            tc.tile_pool(name="const", bufs=1) as const,
            tc.tile_pool(name="xp", bufs=3) as xp,
            tc.tile_pool(name="qkvp", bufs=3) as qkvp,
            tc.tile_pool(name="rtp", bufs=2) as rtp,
            tc.tile_pool(name="bigp", bufs=2) as bigp,
            tc.tile_pool(name="ptp", bufs=8) as ptp,
            tc.tile_pool(name="outp", bufs=3) as outp,
            tc.tile_pool(name="rsp", bufs=2) as rsp,
            tc.tile_pool(name="ps", bufs=2, space="PSUM") as ps,
            tc.tile_pool(name="st", bufs=3, space="PSUM") as stps,
            tc.tile_pool(name="tp", bufs=1, space="PSUM") as tpps,
            tc.tile_pool(name="pc", bufs=2, space="PSUM") as pc,
        ):
            ident = const.tile([128, 128], bf16)
            make_identity(nc, ident)
            wq = const.tile([128, KT16, 3 * SC], bf16)
            nc.sync.dma_start(wq[:], wq_d[:])
            w2 = const.tile([128, 2, C], bf16)
            nc.sync.dma_start(w2[:], w2_d[:])
            br = const.tile([128, 2 * SC], f32)
            nc.sync.dma_start(br[:], br_d[:])
            c1 = const.tile([128, MPB, SC], f32)
            nc.sync.dma_start(c1[:], c1_d[:])
            c2 = const.tile([128, MPB, SC], f32)
            nc.sync.dma_start(c2[:], c2_d[:])
            mk = const.tile([128, 1024], bf16)
            nc.sync.dma_start(mk[:], mk_d[:])

            for b in range(B):
                QT = bigp.tile([128, 2, T], bf16, tag="qt")
                KTt = bigp.tile([128, 2, T], bf16, tag="kt")
                Vp = bigp.tile([128, MPB, HPC, HD + 1], bf16, tag="vp")
                ctxT = bigp.tile([128, 2, T], bf16, tag="ct")
                nc.gpsimd.memset(Vp[:, :, :, HD:HD + 1], 1.0)

                # ---- phase 1: qkv + rope + transpose ----
                for m8 in range(MPB):
                    m = b * MPB + m8
                    xt = xp.tile([128, KT16, 128], bf16)
                    nc.sync.dma_start(
                        xt[:], xt_d[:, m, :].rearrange("p (k t) -> p k t", k=KT16))
                    psA = ps.tile([128, 512], f32, tag="ps")
                    psB = ps.tile([128, 512], f32, tag="ps")
                    for k in range(KT16):
                        nc.tensor.matmul(
                            psA[:], xt[:, k, :], wq[:, k, 0:512],
                            start=(k == 0), stop=(k == KT16 - 1))
                    for k in range(KT16):
                        nc.tensor.matmul(
                            psB[:, 0:256], xt[:, k, :], wq[:, k, 512:768],
                            start=(k == 0), stop=(k == KT16 - 1))
                    # q/k: bias add + rope -> bf16 staging
                    qkv = qkvp.tile([128, 512], f32)
                    nc.vector.tensor_add(qkv[:], psA[:], br[:])
                    qk16 = qkvp.tile([128, 512], bf16, tag="qk16")
                    c1v = c1[:, m8, :].rearrange("p (h d) -> p h d", h=HPC)
                    c2v = c2[:, m8, :].rearrange("p (h d) -> p h d", h=HPC)
                    for base in (0, 256):
                        sec = qkv[:, base:base + 256].rearrange(
                            "p (h d) -> p h d", h=HPC)
                        dst = qk16[:, base:base + 256].rearrange(
                            "p (h d) -> p h d", h=HPC)
                        rt = rtp.tile([128, 256], f32)
                        rtv = rt.rearrange("p (h d) -> p h d", h=HPC)
                        nc.vector.tensor_mul(
                            rtv[:, :, 0:16], sec[:, :, 16:32], c2v[:, :, 0:16])
                        nc.vector.tensor_mul(
                            rtv[:, :, 16:32], sec[:, :, 0:16], c2v[:, :, 16:32])
                        nc.vector.tensor_mul(dst[:], sec[:], c1v)
                        nc.vector.tensor_add(
                            dst[:, :, 0:ROT], dst[:, :, 0:ROT], rtv[:, :, 0:ROT])
                    # v: straight copy into Vp (token-major; bias folded on
                    # host) -- after rope so the PE transposes aren't stalled
                    nc.vector.tensor_copy(
                        Vp[:, m8, :, 0:HD],
                        psB[:, 0:256].rearrange("p (h d) -> p h d", h=HPC))
                    # transpose q/k -> QT/KT (bf16, 1 cyc/row)
                    tp = tpps.tile([128, 1024], bf16)
                    for si, (base, dstT) in enumerate(((0, QT), (256, KTt))):
                        for ci in range(2):
                            nc.tensor.transpose(
                                tp[:, si * 512 + ci * 128:
                                   si * 512 + (ci + 1) * 128],
                                qk16[:, base + ci * 128: base + (ci + 1) * 128],
                                ident)
                        nc.scalar.copy(
                            dstT[:, :, m8 * 128:(m8 + 1) * 128],
                            tp[:, si * 512: si * 512 + 256]
                            .rearrange("p (c t) -> p c t", c=2))

                # ---- phase 2: attention ----
                for h in range(HPC):
                    p0 = (h % 2) * 64
                    qt_h = QT[p0:p0 + 64, h // 2, :]
                    kt_h = KTt[p0:p0 + 64, h // 2, :]
                    for qb in range(2):
                        pct = pc.tile([HD + 1, 512], f32, tag="pc")
                        nst = 4 * (qb + 1)
                        # software pipeline: AV(i) is issued on the PE only
                        # after scores(i+LA), hiding the exp/mask round-trip
                        LA = 3
                        pts = []

                        def do_av(i):
                            nc.tensor.matmul(
                                pct[:], Vp[:, i, h, :], pts[i][:],
                                start=(i == 0), stop=(i == nst - 1))

                        for st in range(nst):
                            stp = stps.tile([128, 512], f32)
                            nc.tensor.matmul(
                                stp[:], kt_h[:, st * 128:(st + 1) * 128],
                                qt_h[:, qb * 512:(qb + 1) * 512],
                                start=True, stop=True)
                            pt = ptp.tile([128, 512], bf16)
                            nc.scalar.activation(
                                pt[:], stp[:],
                                mybir.ActivationFunctionType.Exp)
                            # causal zeroing of the diagonal tiles (0/1 bf16
                            # mask, post-exp, Pool engine: sbuf-only)
                            r = st - 4 * qb
                            if r >= 0:
                                w = 128 * (r + 1)
                                off = 512 - 128 * r
                                nc.gpsimd.tensor_mul(
                                    pt[:, 0:w], pt[:, 0:w], mk[:, off:off + w])
                            pts.append(pt)
                            if st >= LA:
                                do_av(st - LA)
                        for i in range(max(0, nst - LA), nst):
                            do_av(i)
                        # evacuate denom row to sbuf (ACT honors the
                        # partition offset; the custom DVE op does not)
                        rs_s = rsp.tile([1, 512], f32, tag="rss")
                        nc.scalar.copy(rs_s[:], pct[HD:HD + 1, :])
                        rs = rsp.tile([1, 512], f32)
                        nc.vector.reciprocal_approx_fast(rs[:], rs_s[:])
                        rsb = rsp.tile([HD, 512], f32, tag="rsb")
                        nc.gpsimd.partition_broadcast(rsb[:], rs[:])
                        nc.vector.tensor_mul(
                            ctxT[p0:p0 + 64, h // 2, qb * 512:(qb + 1) * 512],
                            pct[0:HD, :], rsb[:])

                # ---- phase 3: out projection partial ----
                for m8 in range(MPB):
                    m = b * MPB + m8
                    ot = outp.tile([128, C], bf16)
                    for n in range(4):
                        po = ps.tile([128, 512], f32, tag="ps")
                        for j in range(2):
                            nc.tensor.matmul(
                                po[:], ctxT[:, j, m8 * 128:(m8 + 1) * 128],
                                w2[:, j, n * 512:(n + 1) * 512],
                                start=(j == 0), stop=(j == 1))
                        if n % 2 == 0:
                            nc.scalar.copy(ot[:, n * 512:(n + 1) * 512], po[:])
                        else:
                            nc.vector.tensor_copy(
                                ot[:, n * 512:(n + 1) * 512], po[:])
                    nc.sync.dma_start(out_d[m, :, :], ot[:])

    nc.finalize()
    return nc


def _host_prep(x, rope, Wqkv_w, Wqkv_b, out_w):
    """Build per-core input maps (partition-first layouts, bf16 matmul ins)."""
    import ml_dtypes
    bf = ml_dtypes.bfloat16

    xf = np.ascontiguousarray(x.reshape(NTOK, C)).astype(np.float32)
    # xt[p, m, k*128 + t] = x[m*128+t, k*128+p]
    xt = np.ascontiguousarray(
        xf.reshape(MT, 128, KT16, 128).transpose(3, 0, 2, 1)
        .reshape(128, MT, KT16 * 128)).astype(bf)

    # rope tables (position within a batch: t = 0..1023)
    cos = rope[:, :, 0].astype(np.float32)   # [T, 16]
    sin = rope[:, :, 1].astype(np.float32)
    C1h = np.ones((T, HD), np.float32)
    C1h[:, 0:16] = cos
    C1h[:, 16:32] = cos
    C2h = np.zeros((T, HD), np.float32)
    C2h[:, 0:16] = -sin
    C2h[:, 16:32] = sin
    C1 = np.tile(C1h, (1, HPC))              # [T, 256]
    C2 = np.tile(C2h, (1, HPC))
    # c1[p, q, j] = C1[q*128+p, j]
    c1 = np.ascontiguousarray(C1.reshape(MPB, 128, SC).transpose(1, 0, 2))
    c2 = np.ascontiguousarray(C2.reshape(MPB, 128, SC).transpose(1, 0, 2))

    # causal keep-mask table: mk[p, y] = 0 if y < p + 512 else 1
    yy = np.arange(1024)[None, :]
    pp = np.arange(128)[:, None]
    mk = np.where(yy < pp + 512, 0.0, 1.0).astype(bf)

    scale = np.float32(1.0 / np.sqrt(HD))
    in_maps = []
    for g in range(NCORES):
        hs = g * SC
        Wq = Wqkv_w[hs:hs + SC, :].astype(np.float32) * scale
        Wk = Wqkv_w[C + hs:C + hs + SC, :].astype(np.float32)
        Wv = Wqkv_w[2 * C + hs:2 * C + hs + SC, :].astype(np.float32)
        Wsh = np.concatenate([Wq, Wk, Wv], axis=0)          # [768, 2048]
        # wq[p, k, j] = Wsh[j, k*128+p]
        wqa = np.ascontiguousarray(
            Wsh.T.reshape(KT16, 128, 3 * SC).transpose(1, 0, 2)).astype(bf)
        bq = Wqkv_b[hs:hs + SC].astype(np.float32) * scale
        bk = Wqkv_b[C + hs:C + hs + SC].astype(np.float32)
        bsh = np.concatenate([bq, bk])
        bra = np.ascontiguousarray(np.broadcast_to(bsh, (128, 2 * SC)))
        # w2[p, j, o] = out_w[o, g*256 + j*128 + p]
        w2a = np.ascontiguousarray(
            out_w[:, hs:hs + SC].astype(np.float32).T.reshape(
                2, 128, C).transpose(1, 0, 2)).astype(bf)
        in_maps.append({
            "xt": xt, "wq": wqa, "br": bra, "c1": c1, "c2": c2,
            "mk": mk, "w2": w2a,
        })
    return in_maps


def kernel(x, mask, index, rope, Wqkv_w, Wqkv_b, out_w, out_b,
           k_cache, v_cache):
    from concourse.bass_utils import run_bass_kernel_spmd

    x = np.asarray(x)
    rope = np.asarray(rope)
    Wqkv_w = np.asarray(Wqkv_w)
    Wqkv_b = np.asarray(Wqkv_b)
    out_w = np.asarray(out_w)
    out_b = np.asarray(out_b)

    if "nc" not in _CACHE:
        _CACHE["nc"] = _build_nc()
    nc = _CACHE["nc"]

    in_maps = _host_prep(x, rope, Wqkv_w, Wqkv_b, out_w)
    res = run_bass_kernel_spmd(nc, in_maps, core_ids=list(range(NCORES)))

    acc = np.zeros((NTOK, C), np.float32)
    for g in range(NCORES):
        acc += res.results[g]["out"].reshape(NTOK, C).astype(np.float32)
    # out bias + v-bias folded through the output projection
    bv = Wqkv_b[2 * C:3 * C].astype(np.float32)
    acc += out_b.astype(np.float32) + bv @ out_w.astype(np.float32).T
    return acc.reshape(B, T, C)


# revision 28
# speedup vs baseline: 1.6709x; 1.6709x over previous
"""Trainium2 Bass kernel for nn_MHA_9603546874182.

Causal MHA: qkv proj + rope(32) + causal attention + out proj.
B=4, T=1024, C=2048, H=32, hd=64.

Sharding: 8-way tensor parallel over heads (4 heads / core).
Each core computes qkv for its 4 heads (column-parallel), rope,
causal attention, and a row-parallel partial of the output
projection. Host sums the 8 bf16 partials (+ bias, incl. the v-bias
contribution folded through out_w).

All matmuls run in bf16 (1 cycle/row on the PE at 2.4 GHz vs the
2-4x slower fp32r path measured on HW). Work is spread across
engines: DVE does rope/bias/normalize, ACT does exp + the q/k
transpose-psum evacuations, Pool (gpsimd) does mask adds + psum->
sbuf copies for v and the output, PE does matmuls + transposes.
"""

import numpy as np

B, T, C, H = 4, 1024, 2048, 32
HD = C // H          # 64
NCORES = 8
HPC = H // NCORES    # 4 heads per core
SC = HPC * HD        # 256 shard channels
NTOK = B * T         # 4096
KT16 = C // 128      # 16 k tiles
MT = NTOK // 128     # 32 token tiles
MPB = T // 128       # 8 token tiles per batch
ROT = 32
NEG = -1.0e9

_CACHE = {}


def _build_nc():
    import concourse.bass as bass
    import concourse.mybir as mybir
    import concourse.tile as tile
    from concourse import bacc
    from concourse.masks import make_identity

    f32 = mybir.dt.float32
    f32r = mybir.dt.float32r
    bf16 = mybir.dt.bfloat16

    nc = bacc.Bacc("TRN2")

    xt_d = nc.dram_tensor("xt", [128, MT, KT16 * 128], bf16, kind="ExternalInput")
    wq_d = nc.dram_tensor("wq", [128, KT16, 3 * SC], bf16, kind="ExternalInput")
    br_d = nc.dram_tensor("br", [128, 2 * SC], f32, kind="ExternalInput")
    c1_d = nc.dram_tensor("c1", [128, MPB, SC], f32, kind="ExternalInput")
    c2_d = nc.dram_tensor("c2", [128, MPB, SC], f32, kind="ExternalInput")
    mk_d = nc.dram_tensor("mk", [128, 1024], bf16, kind="ExternalInput")
    w2_d = nc.dram_tensor("w2", [128, 2, C], bf16, kind="ExternalInput")
    out_d = nc.dram_tensor("out", [MT, 128, C], bf16, kind="ExternalOutput")

    with tile.TileContext(nc) as tc:
        with (
            tc.tile_pool(name="const", bufs=1) as const,
            tc.tile_pool(name="xp", bufs=3) as xp,
            tc.tile_pool(name="qkvp", bufs=3) as qkvp,
            tc.tile_pool(name="rtp", bufs=2) as rtp,
            tc.tile_pool(name="bigp", bufs=2) as bigp,
            tc.tile_pool(name="ptp", bufs=8) as ptp,
            tc.tile_pool(name="outp", bufs=3) as outp,
            tc.tile_pool(name="rsp", bufs=2) as rsp,
            tc.tile_pool(name="ps", bufs=2, space="PSUM") as ps,
            tc.tile_pool(name="st", bufs=3, space="PSUM") as stps,
            tc.tile_pool(name="tp", bufs=1, space="PSUM") as tpps,
            tc.tile_pool(name="pc", bufs=2, space="PSUM") as pc,
        ):
            ident = const.tile([128, 128], bf16)
            make_identity(nc, ident)
            ones64 = const.tile([1, 64], f32)
            nc.vector.memset(ones64[:], 1.0)
            wq = const.tile([128, KT16, 3 * SC], bf16)
            nc.sync.dma_start(wq[:], wq_d[:])
            w2 = const.tile([128, 2, C], bf16)
            nc.sync.dma_start(w2[:], w2_d[:])
            br = const.tile([128, 2 * SC], f32)
            nc.sync.dma_start(br[:], br_d[:])
            c1 = const.tile([128, MPB, SC], f32)
            nc.sync.dma_start(c1[:], c1_d[:])
            c2 = const.tile([128, MPB, SC], f32)
            nc.sync.dma_start(c2[:], c2_d[:])
            mk = const.tile([128, 1024], bf16)
            nc.sync.dma_start(mk[:], mk_d[:])

            for b in range(B):
                QT = bigp.tile([128, 2, T], bf16, tag="qt")
                KTt = bigp.tile([128, 2, T], bf16, tag="kt")
                Vp = bigp.tile([128, MPB, HPC, HD + 1], bf16, tag="vp")
                ctxT = bigp.tile([128, 2, T], bf16, tag="ct")
                nc.vector.memset(Vp[:, :, :, HD:HD + 1], 1.0)

                # ---- phase 1: qkv + rope + transpose ----
                for m8 in range(MPB):
                    m = b * MPB + m8
                    xt = xp.tile([128, KT16, 128], bf16)
                    nc.sync.dma_start(
                        xt[:], xt_d[:, m, :].rearrange("p (k t) -> p k t", k=KT16))
                    psA = ps.tile([128, 512], f32, tag="ps")
                    psB = ps.tile([128, 512], f32, tag="ps")
                    for k in range(KT16):
                        nc.tensor.matmul(
                            psA[:], xt[:, k, :], wq[:, k, 0:512],
                            start=(k == 0), stop=(k == KT16 - 1))
                    for k in range(KT16):
                        nc.tensor.matmul(
                            psB[:, 0:256], xt[:, k, :], wq[:, k, 512:768],
                            start=(k == 0), stop=(k == KT16 - 1))
                    # q/k: bias add + rope -> bf16 staging
                    qkv = qkvp.tile([128, 512], f32)
                    nc.vector.tensor_add(qkv[:], psA[:], br[:])
                    qk16 = qkvp.tile([128, 512], bf16, tag="qk16")
                    c1v = c1[:, m8, :].rearrange("p (h d) -> p h d", h=HPC)
                    c2v = c2[:, m8, :].rearrange("p (h d) -> p h d", h=HPC)
                    for base in (0, 256):
                        sec = qkv[:, base:base + 256].rearrange(
                            "p (h d) -> p h d", h=HPC)
                        dst = qk16[:, base:base + 256].rearrange(
                            "p (h d) -> p h d", h=HPC)
                        rt = rtp.tile([128, 256], f32)
                        rtv = rt.rearrange("p (h d) -> p h d", h=HPC)
                        nc.vector.tensor_mul(
                            rtv[:, :, 0:16], sec[:, :, 16:32], c2v[:, :, 0:16])
                        nc.vector.tensor_mul(
                            rtv[:, :, 16:32], sec[:, :, 0:16], c2v[:, :, 16:32])
                        nc.vector.tensor_mul(dst[:], sec[:], c1v)
                        nc.vector.tensor_add(
                            dst[:, :, 0:ROT], dst[:, :, 0:ROT], rtv[:, :, 0:ROT])
                    # v: straight copy into Vp (token-major; bias folded on
                    # host) -- after rope so the PE transposes aren't stalled
                    nc.vector.tensor_copy(
                        Vp[:, m8, :, 0:HD],
                        psB[:, 0:256].rearrange("p (h d) -> p h d", h=HPC))
                    # transpose q/k -> QT/KT (bf16, 1 cyc/row)
                    tp = tpps.tile([128, 1024], bf16)
                    for si, (base, dstT) in enumerate(((0, QT), (256, KTt))):
                        for ci in range(2):
                            nc.tensor.transpose(
                                tp[:, si * 512 + ci * 128:
                                   si * 512 + (ci + 1) * 128],
                                qk16[:, base + ci * 128: base + (ci + 1) * 128],
                                ident)
                        nc.scalar.copy(
                            dstT[:, :, m8 * 128:(m8 + 1) * 128],
                            tp[:, si * 512: si * 512 + 256]
                            .rearrange("p (c t) -> p c t", c=2))

                # ---- phase 2: attention ----
                # Group tails (normalize + ctx write) are deferred into the
                # next group's scores stream so the PE never waits on the
                # ACT-copy -> reciprocal chain.
                pending_tail = [None]

                def flush_tail():
                    if pending_tail[0] is not None:
                        pending_tail[0]()
                        pending_tail[0] = None

                for h in range(HPC):
                    p0 = (h % 2) * 64
                    qt_h = QT[p0:p0 + 64, h // 2, :]
                    kt_h = KTt[p0:p0 + 64, h // 2, :]
                    for qb in range(2):
                        pct = pc.tile([128, 512], f32, tag="pc")
                        nst = 4 * (qb + 1)
                        # software pipeline: AV(i) is issued on the PE only
                        # after scores(i+LA), hiding the exp/mask round-trip
                        LA = 3
                        pts = []

                        def do_av(i, pct=pct, nst=nst, h=h, pts=pts):
                            nc.tensor.matmul(
                                pct[0:HD + 1, :], Vp[:, i, h, :], pts[i][:],
                                start=(i == 0), stop=(i == nst - 1))

                        for st in range(nst):
                            stp = stps.tile([128, 512], f32)
                            nc.tensor.matmul(
                                stp[:], kt_h[:, st * 128:(st + 1) * 128],
                                qt_h[:, qb * 512:(qb + 1) * 512],
                                start=True, stop=True)
                            pt = ptp.tile([128, 512], bf16)
                            nc.scalar.activation(
                                pt[:], stp[:],
                                mybir.ActivationFunctionType.Exp)
                            # causal zeroing of the diagonal tiles (0/1 bf16
                            # mask, post-exp, Pool engine: sbuf-only)
                            r = st - 4 * qb
                            if r >= 0:
                                w = 128 * (r + 1)
                                off = 512 - 128 * r
                                nc.gpsimd.tensor_mul(
                                    pt[:, 0:w], pt[:, 0:w], mk[:, off:off + w])
                            pts.append(pt)
                            if st == LA:
                                # previous group's tail: its reciprocal is
                                # ready by now; PE does a 213ns broadcast
                                flush_tail()
                            if st >= LA:
                                do_av(st - LA)
                        for i in range(max(0, nst - LA), nst):
                            do_av(i)
                        # evacuate denom row to sbuf (ACT honors the
                        # partition offset; the custom DVE op does not)
                        rs_s = rsp.tile([1, 512], f32, tag="rss")
                        nc.scalar.copy(rs_s[:], pct[HD:HD + 1, :])
                        rs = rsp.tile([1, 512], f32)
                        nc.vector.reciprocal_approx_fast(rs[:], rs_s[:])

                        def tail(pct=pct, rs=rs, h=h, qb=qb, p0=p0):
                            # broadcast 1/denom across 64 partitions via a
                            # rank-1 PE matmul into the spare upper half of
                            # the pct bank (avoids the GpSimd ucode-library
                            # thrash of partition_broadcast)
                            nc.tensor.matmul(
                                pct[64:128, :], ones64[:], rs[:],
                                start=True, stop=True, skip_group_check=True)
                            rsb = rsp.tile([HD, 512], bf16, tag="rsb")
                            nc.scalar.copy(rsb[:], pct[64:128, :])
                            nc.vector.tensor_mul(
                                ctxT[p0:p0 + 64, h // 2,
                                     qb * 512:(qb + 1) * 512],
                                pct[0:HD, :], rsb[:])

                        pending_tail[0] = tail
                flush_tail()

                # ---- phase 3: out projection partial ----
                for m8 in range(MPB):
                    m = b * MPB + m8
                    ot = outp.tile([128, C], bf16)
                    for n in range(4):
                        po = ps.tile([128, 512], f32, tag="ps")
                        for j in range(2):
                            nc.tensor.matmul(
                                po[:], ctxT[:, j, m8 * 128:(m8 + 1) * 128],
                                w2[:, j, n * 512:(n + 1) * 512],
                                start=(j == 0), stop=(j == 1))
                        if n % 2 == 0:
                            nc.scalar.copy(ot[:, n * 512:(n + 1) * 512], po[:])
                        else:
                            nc.vector.tensor_copy(
                                ot[:, n * 512:(n + 1) * 512], po[:])
                    nc.sync.dma_start(out_d[m, :, :], ot[:])

    nc.finalize()
    return nc


def _host_prep(x, rope, Wqkv_w, Wqkv_b, out_w):
    """Build per-core input maps (partition-first layouts, bf16 matmul ins)."""
    import ml_dtypes
    bf = ml_dtypes.bfloat16

    xf = np.ascontiguousarray(x.reshape(NTOK, C)).astype(np.float32)
    # xt[p, m, k*128 + t] = x[m*128+t, k*128+p]
    xt = np.ascontiguousarray(
        xf.reshape(MT, 128, KT16, 128).transpose(3, 0, 2, 1)
        .reshape(128, MT, KT16 * 128)).astype(bf)

    # rope tables (position within a batch: t = 0..1023)
    cos = rope[:, :, 0].astype(np.float32)   # [T, 16]
    sin = rope[:, :, 1].astype(np.float32)
    C1h = np.ones((T, HD), np.float32)
    C1h[:, 0:16] = cos
    C1h[:, 16:32] = cos
    C2h = np.zeros((T, HD), np.float32)
    C2h[:, 0:16] = -sin
    C2h[:, 16:32] = sin
    C1 = np.tile(C1h, (1, HPC))              # [T, 256]
    C2 = np.tile(C2h, (1, HPC))
    # c1[p, q, j] = C1[q*128+p, j]
    c1 = np.ascontiguousarray(C1.reshape(MPB, 128, SC).transpose(1, 0, 2))
    c2 = np.ascontiguousarray(C2.reshape(MPB, 128, SC).transpose(1, 0, 2))

    # causal keep-mask table: mk[p, y] = 0 if y < p + 512 else 1
    yy = np.arange(1024)[None, :]
    pp = np.arange(128)[:, None]
    mk = np.where(yy < pp + 512, 0.0, 1.0).astype(bf)

    scale = np.float32(1.0 / np.sqrt(HD))
    in_maps = []
    for g in range(NCORES):
        hs = g * SC
        Wq = Wqkv_w[hs:hs + SC, :].astype(np.float32) * scale
        Wk = Wqkv_w[C + hs:C + hs + SC, :].astype(np.float32)
        Wv = Wqkv_w[2 * C + hs:2 * C + hs + SC, :].astype(np.float32)
        Wsh = np.concatenate([Wq, Wk, Wv], axis=0)          # [768, 2048]
        # wq[p, k, j] = Wsh[j, k*128+p]
        wqa = np.ascontiguousarray(
            Wsh.T.reshape(KT16, 128, 3 * SC).transpose(1, 0, 2)).astype(bf)
        bq = Wqkv_b[hs:hs + SC].astype(np.float32) * scale
        bk = Wqkv_b[C + hs:C + hs + SC].astype(np.float32)
        bsh = np.concatenate([bq, bk])
        bra = np.ascontiguousarray(np.broadcast_to(bsh, (128, 2 * SC)))
        # w2[p, j, o] = out_w[o, g*256 + j*128 + p]
        w2a = np.ascontiguousarray(
            out_w[:, hs:hs + SC].astype(np.float32).T.reshape(
                2, 128, C).transpose(1, 0, 2)).astype(bf)
        in_maps.append({
            "xt": xt, "wq": wqa, "br": bra, "c1": c1, "c2": c2,
            "mk": mk, "w2": w2a,
        })
    return in_maps


def kernel(x, mask, index, rope, Wqkv_w, Wqkv_b, out_w, out_b,
           k_cache, v_cache):
    from concourse.bass_utils import run_bass_kernel_spmd

    x = np.asarray(x)
    rope = np.asarray(rope)
    Wqkv_w = np.asarray(Wqkv_w)
    Wqkv_b = np.asarray(Wqkv_b)
    out_w = np.asarray(out_w)
    out_b = np.asarray(out_b)

    if "nc" not in _CACHE:
        _CACHE["nc"] = _build_nc()
    nc = _CACHE["nc"]

    in_maps = _host_prep(x, rope, Wqkv_w, Wqkv_b, out_w)
    res = run_bass_kernel_spmd(nc, in_maps, core_ids=list(range(NCORES)))

    acc = np.zeros((NTOK, C), np.float32)
    for g in range(NCORES):
        acc += res.results[g]["out"].reshape(NTOK, C).astype(np.float32)
    # out bias + v-bias folded through the output projection
    bv = Wqkv_b[2 * C:3 * C].astype(np.float32)
    acc += out_b.astype(np.float32) + bv @ out_w.astype(np.float32).T
    return acc.reshape(B, T, C)


# revision 33
# speedup vs baseline: 1.8587x; 1.1124x over previous
"""Trainium2 Bass kernel for nn_MHA_9603546874182.

Causal MHA: qkv proj + rope(32) + causal attention + out proj.
B=4, T=1024, C=2048, H=32, hd=64.

Sharding: 8-way tensor parallel over heads (4 heads / core).
Each core computes qkv for its 4 heads (column-parallel), rope,
causal attention, and a row-parallel partial of the output
projection. Host sums the 8 bf16 partials (+ bias, incl. the v-bias
contribution folded through out_w).

All matmuls run in bf16 (1 cycle/row on the PE at 2.4 GHz vs the
2-4x slower fp32r path measured on HW). Work is spread across
engines: DVE does rope/bias/normalize, ACT does exp + the q/k
transpose-psum evacuations, Pool (gpsimd) does mask adds + psum->
sbuf copies for v and the output, PE does matmuls + transposes.
"""

import numpy as np

B, T, C, H = 4, 1024, 2048, 32
HD = C // H          # 64
NCORES = 8
HPC = H // NCORES    # 4 heads per core
SC = HPC * HD        # 256 shard channels
NTOK = B * T         # 4096
KT16 = C // 128      # 16 k tiles
MT = NTOK // 128     # 32 token tiles
MPB = T // 128       # 8 token tiles per batch
ROT = 32
NEG = -1.0e9

_CACHE = {}


def _build_nc():
    import concourse.bass as bass
    import concourse.mybir as mybir
    import concourse.tile as tile
    from concourse import bacc
    from concourse.masks import make_identity

    f32 = mybir.dt.float32
    f32r = mybir.dt.float32r
    bf16 = mybir.dt.bfloat16

    nc = bacc.Bacc("TRN2")

    xt_d = nc.dram_tensor("xt", [128, MT, KT16 * 128], bf16, kind="ExternalInput")
    wq_d = nc.dram_tensor("wq", [128, KT16, 3 * SC], bf16, kind="ExternalInput")
    br_d = nc.dram_tensor("br", [128, 2 * SC], f32, kind="ExternalInput")
    c1_d = nc.dram_tensor("c1", [128, MPB, SC], f32, kind="ExternalInput")
    c2_d = nc.dram_tensor("c2", [128, MPB, SC], f32, kind="ExternalInput")
    mk_d = nc.dram_tensor("mk", [128, 1024], bf16, kind="ExternalInput")
    w2_d = nc.dram_tensor("w2", [128, 2, C], bf16, kind="ExternalInput")
    out_d = nc.dram_tensor("out", [MT, 128, C], bf16, kind="ExternalOutput")

    with tile.TileContext(nc) as tc:
        with (
            tc.tile_pool(name="const", bufs=1) as const,
            tc.tile_pool(name="xp", bufs=3) as xp,
            tc.tile_pool(name="qkvp", bufs=3) as qkvp,
            tc.tile_pool(name="rtp", bufs=2) as rtp,
            tc.tile_pool(name="bigp", bufs=2) as bigp,
            tc.tile_pool(name="ptp", bufs=12) as ptp,
            tc.tile_pool(name="outp", bufs=3) as outp,
            tc.tile_pool(name="rsp", bufs=2) as rsp,
            tc.tile_pool(name="ps", bufs=5, space="PSUM") as ps,
            tc.tile_pool(name="tp", bufs=1, space="PSUM") as tpps,
            tc.tile_pool(name="pc", bufs=2, space="PSUM") as pc,
        ):
            ident = const.tile([128, 128], bf16)
            make_identity(nc, ident)
            ones64 = const.tile([1, 64], bf16)
            nc.vector.memset(ones64[:], 1.0)
            wq = const.tile([128, KT16, 3 * SC], bf16)
            nc.sync.dma_start(wq[:], wq_d[:])
            w2 = const.tile([128, 2, C], bf16)
            nc.sync.dma_start(w2[:], w2_d[:])
            br = const.tile([128, 2 * SC], f32)
            nc.sync.dma_start(br[:], br_d[:])
            c1 = const.tile([128, MPB, SC], f32)
            nc.sync.dma_start(c1[:], c1_d[:])
            c2 = const.tile([128, MPB, SC], f32)
            nc.sync.dma_start(c2[:], c2_d[:])
            mk = const.tile([128, 1024], bf16)
            nc.sync.dma_start(mk[:], mk_d[:])

            for b in range(B):
                QT = bigp.tile([128, 2, T], bf16, tag="qt")
                KTt = bigp.tile([128, 2, T], bf16, tag="kt")
                Vp = bigp.tile([128, MPB, HPC, HD + 1], bf16, tag="vp")
                ctxT = bigp.tile([128, 2, T], bf16, tag="ct")
                nc.vector.memset(Vp[:, :, :, HD:HD + 1], 1.0)

                # ---- phase 1: qkv + rope + transpose ----
                for m8 in range(MPB):
                    m = b * MPB + m8
                    xt = xp.tile([128, KT16, 128], bf16)
                    nc.sync.dma_start(
                        xt[:], xt_d[:, m, :].rearrange("p (k t) -> p k t", k=KT16))
                    psA = ps.tile([128, 512], f32, tag="ps")
                    psB = ps.tile([128, 512], f32, tag="ps")
                    for k in range(KT16):
                        nc.tensor.matmul(
                            psA[:], xt[:, k, :], wq[:, k, 0:512],
                            start=(k == 0), stop=(k == KT16 - 1))
                    for k in range(KT16):
                        nc.tensor.matmul(
                            psB[:, 0:256], xt[:, k, :], wq[:, k, 512:768],
                            start=(k == 0), stop=(k == KT16 - 1))
                    # q/k: bias add + rope -> bf16 staging
                    qkv = qkvp.tile([128, 512], f32)
                    nc.vector.tensor_add(qkv[:], psA[:], br[:])
                    qk16 = qkvp.tile([128, 512], bf16, tag="qk16")
                    c1v = c1[:, m8, :].rearrange("p (h d) -> p h d", h=HPC)
                    c2v = c2[:, m8, :].rearrange("p (h d) -> p h d", h=HPC)
                    for base in (0, 256):
                        sec = qkv[:, base:base + 256].rearrange(
                            "p (h d) -> p h d", h=HPC)
                        dst = qk16[:, base:base + 256].rearrange(
                            "p (h d) -> p h d", h=HPC)
                        rt = rtp.tile([128, 256], f32)
                        rtv = rt.rearrange("p (h d) -> p h d", h=HPC)
                        nc.vector.tensor_mul(
                            rtv[:, :, 0:16], sec[:, :, 16:32], c2v[:, :, 0:16])
                        nc.vector.tensor_mul(
                            rtv[:, :, 16:32], sec[:, :, 0:16], c2v[:, :, 16:32])
                        nc.vector.tensor_mul(dst[:], sec[:], c1v)
                        nc.vector.tensor_add(
                            dst[:, :, 0:ROT], dst[:, :, 0:ROT], rtv[:, :, 0:ROT])
                    # v: straight copy into Vp (token-major; bias folded on
                    # host) -- after rope so the PE transposes aren't stalled
                    nc.vector.tensor_copy(
                        Vp[:, m8, :, 0:HD],
                        psB[:, 0:256].rearrange("p (h d) -> p h d", h=HPC))
                    # transpose q/k -> QT/KT (bf16, 1 cyc/row)
                    tp = tpps.tile([128, 1024], bf16)
                    for si, (base, dstT) in enumerate(((0, QT), (256, KTt))):
                        for ci in range(2):
                            nc.tensor.transpose(
                                tp[:, si * 512 + ci * 128:
                                   si * 512 + (ci + 1) * 128],
                                qk16[:, base + ci * 128: base + (ci + 1) * 128],
                                ident)
                        nc.scalar.copy(
                            dstT[:, :, m8 * 128:(m8 + 1) * 128],
                            tp[:, si * 512: si * 512 + 256]
                            .rearrange("p (c t) -> p c t", c=2))

                # ---- phase 2: attention ----
                # Two qb-groups per head are interleaved on the PE so the
                # exp/mask round-trip of one group hides behind the other's
                # matmuls (keeps the PE dense -> full p-state clock).
                # Normalize tails are likewise deferred into the next head's
                # score stream.
                pending_tails = []

                def emit_score(g):
                    st = g["scored"]
                    stp = ps.tile([128, 512], f32, tag="ps")
                    nc.tensor.matmul(
                        stp[:], g["kt"][:, st * 128:(st + 1) * 128],
                        g["qt"][:, g["qb"] * 512:(g["qb"] + 1) * 512],
                        start=True, stop=True)
                    pt = ptp.tile([128, 512], bf16)
                    nc.scalar.activation(
                        pt[:], stp[:], mybir.ActivationFunctionType.Exp)
                    # causal zeroing of diagonal tiles (0/1 bf16 mask,
                    # post-exp, Pool engine: sbuf-only)
                    r = st - 4 * g["qb"]
                    if r >= 0:
                        w = 128 * (r + 1)
                        off = 512 - 128 * r
                        nc.gpsimd.tensor_mul(
                            pt[:, 0:w], pt[:, 0:w], mk[:, off:off + w])
                    g["pts"].append(pt)
                    g["scored"] += 1

                def emit_av(g):
                    i = g["av"]
                    nc.tensor.matmul(
                        g["pct"][0:HD + 1, :], Vp[:, i, g["h"], :],
                        g["pts"][i][:],
                        start=(i == 0), stop=(i == g["nst"] - 1))
                    g["av"] += 1

                def emit_tail_pre(g):
                    # denom row -> sbuf in bf16 (ACT honors the partition
                    # offset; the custom DVE reciprocal does not)
                    rs_s = rsp.tile([1, 512], bf16, tag="rss")
                    nc.scalar.copy(rs_s[:], g["pct"][HD:HD + 1, :])
                    g["rs_s"] = rs_s

                def make_tail(g):
                    def tail():
                        # broadcast denom across 64 partitions via a rank-1
                        # bf16 PE matmul into the spare upper half of the
                        # pct bank (avoids GpSimd ucode-library thrash of
                        # partition_broadcast), then reciprocal + ctx write
                        nc.tensor.matmul(
                            g["pct"][64:128, :], ones64[:], g["rs_s"][:],
                            start=True, stop=True, skip_group_check=True)
                        den = rsp.tile([HD, 512], f32, tag="den")
                        nc.scalar.copy(den[:], g["pct"][64:128, :])
                        rsb = rsp.tile([HD, 512], f32, tag="rsb")
                        nc.vector.reciprocal_approx_fast(rsb[:], den[:])
                        nc.vector.tensor_mul(
                            ctxT[g["p0"]:g["p0"] + 64, g["h"] // 2,
                                 g["qb"] * 512:(g["qb"] + 1) * 512],
                            g["pct"][0:HD, :], rsb[:])
                    return tail

                LA = 2
                for h in range(HPC):
                    p0 = (h % 2) * 64
                    qt_h = QT[p0:p0 + 64, h // 2, :]
                    kt_h = KTt[p0:p0 + 64, h // 2, :]
                    groups = []
                    for qb in range(2):
                        pct = pc.tile([128, 512], f32, tag="pc")
                        groups.append(dict(
                            qb=qb, h=h, p0=p0, qt=qt_h, kt=kt_h, pct=pct,
                            nst=4 * (qb + 1), pts=[], scored=0, av=0))
                    emitted = 0
                    while any(g["scored"] < g["nst"] for g in groups):
                        for g in groups:
                            if g["scored"] < g["nst"]:
                                emit_score(g)
                                emitted += 1
                                if emitted in (2, 4) and pending_tails:
                                    pending_tails.pop(0)()
                        for g in groups:
                            if (g["av"] < g["nst"]
                                    and g["av"] <= g["scored"] - 1 - LA):
                                emit_av(g)
                    for g in groups:
                        while g["av"] < g["nst"]:
                            emit_av(g)
                        emit_tail_pre(g)
                        pending_tails.append(make_tail(g))
                while pending_tails:
                    pending_tails.pop(0)()

                # ---- phase 3: out projection partial ----
                for m8 in range(MPB):
                    m = b * MPB + m8
                    ot = outp.tile([128, C], bf16)
                    for n in range(4):
                        po = ps.tile([128, 512], f32, tag="ps")
                        for j in range(2):
                            nc.tensor.matmul(
                                po[:], ctxT[:, j, m8 * 128:(m8 + 1) * 128],
                                w2[:, j, n * 512:(n + 1) * 512],
                                start=(j == 0), stop=(j == 1))
                        if n % 2 == 0:
                            nc.scalar.copy(ot[:, n * 512:(n + 1) * 512], po[:])
                        else:
                            nc.vector.tensor_copy(
                                ot[:, n * 512:(n + 1) * 512], po[:])
                    nc.sync.dma_start(out_d[m, :, :], ot[:])

    nc.finalize()
    return nc


def _host_prep(x, rope, Wqkv_w, Wqkv_b, out_w):
    """Build per-core input maps (partition-first layouts, bf16 matmul ins)."""
    import ml_dtypes
    bf = ml_dtypes.bfloat16

    xf = np.ascontiguousarray(x.reshape(NTOK, C)).astype(np.float32)
    # xt[p, m, k*128 + t] = x[m*128+t, k*128+p]
    xt = np.ascontiguousarray(
        xf.reshape(MT, 128, KT16, 128).transpose(3, 0, 2, 1)
        .reshape(128, MT, KT16 * 128)).astype(bf)

    # rope tables (position within a batch: t = 0..1023)
    cos = rope[:, :, 0].astype(np.float32)   # [T, 16]
    sin = rope[:, :, 1].astype(np.float32)
    C1h = np.ones((T, HD), np.float32)
    C1h[:, 0:16] = cos
    C1h[:, 16:32] = cos
    C2h = np.zeros((T, HD), np.float32)
    C2h[:, 0:16] = -sin
    C2h[:, 16:32] = sin
    C1 = np.tile(C1h, (1, HPC))              # [T, 256]
    C2 = np.tile(C2h, (1, HPC))
    # c1[p, q, j] = C1[q*128+p, j]
    c1 = np.ascontiguousarray(C1.reshape(MPB, 128, SC).transpose(1, 0, 2))
    c2 = np.ascontiguousarray(C2.reshape(MPB, 128, SC).transpose(1, 0, 2))

    # causal keep-mask table: mk[p, y] = 0 if y < p + 512 else 1
    yy = np.arange(1024)[None, :]
    pp = np.arange(128)[:, None]
    mk = np.where(yy < pp + 512, 0.0, 1.0).astype(bf)

    scale = np.float32(1.0 / np.sqrt(HD))
    in_maps = []
    for g in range(NCORES):
        hs = g * SC
        Wq = Wqkv_w[hs:hs + SC, :].astype(np.float32) * scale
        Wk = Wqkv_w[C + hs:C + hs + SC, :].astype(np.float32)
        Wv = Wqkv_w[2 * C + hs:2 * C + hs + SC, :].astype(np.float32)
        Wsh = np.concatenate([Wq, Wk, Wv], axis=0)          # [768, 2048]
        # wq[p, k, j] = Wsh[j, k*128+p]
        wqa = np.ascontiguousarray(
            Wsh.T.reshape(KT16, 128, 3 * SC).transpose(1, 0, 2)).astype(bf)
        bq = Wqkv_b[hs:hs + SC].astype(np.float32) * scale
        bk = Wqkv_b[C + hs:C + hs + SC].astype(np.float32)
        bsh = np.concatenate([bq, bk])
        bra = np.ascontiguousarray(np.broadcast_to(bsh, (128, 2 * SC)))
        # w2[p, j, o] = out_w[o, g*256 + j*128 + p]
        w2a = np.ascontiguousarray(
            out_w[:, hs:hs + SC].astype(np.float32).T.reshape(
                2, 128, C).transpose(1, 0, 2)).astype(bf)
        in_maps.append({
            "xt": xt, "wq": wqa, "br": bra, "c1": c1, "c2": c2,
            "mk": mk, "w2": w2a,
        })
    return in_maps


def kernel(x, mask, index, rope, Wqkv_w, Wqkv_b, out_w, out_b,
           k_cache, v_cache):
    from concourse.bass_utils import run_bass_kernel_spmd

    x = np.asarray(x)
    rope = np.asarray(rope)
    Wqkv_w = np.asarray(Wqkv_w)
    Wqkv_b = np.asarray(Wqkv_b)
    out_w = np.asarray(out_w)
    out_b = np.asarray(out_b)

    if "nc" not in _CACHE:
        _CACHE["nc"] = _build_nc()
    nc = _CACHE["nc"]

    in_maps = _host_prep(x, rope, Wqkv_w, Wqkv_b, out_w)
    res = run_bass_kernel_spmd(nc, in_maps, core_ids=list(range(NCORES)))

    acc = np.zeros((NTOK, C), np.float32)
    for g in range(NCORES):
        acc += res.results[g]["out"].reshape(NTOK, C).astype(np.float32)
    # out bias + v-bias folded through the output projection
    bv = Wqkv_b[2 * C:3 * C].astype(np.float32)
    acc += out_b.astype(np.float32) + bv @ out_w.astype(np.float32).T
    return acc.reshape(B, T, C)


# revision 35
# speedup vs baseline: 2.0414x; 1.0983x over previous
"""Trainium2 Bass kernel for nn_MHA_9603546874182.

Causal MHA: qkv proj + rope(32) + causal attention + out proj.
B=4, T=1024, C=2048, H=32, hd=64.

Sharding: 8-way tensor parallel over heads (4 heads / core).
Each core computes qkv for its 4 heads (column-parallel), rope,
causal attention, and a row-parallel partial of the output
projection. Host sums the 8 bf16 partials (+ bias, incl. the v-bias
contribution folded through out_w).

All matmuls run in bf16 (1 cycle/row on the PE at 2.4 GHz vs the
2-4x slower fp32r path measured on HW). Work is spread across
engines: DVE does rope/bias/normalize, ACT does exp + the q/k
transpose-psum evacuations, Pool (gpsimd) does mask adds + psum->
sbuf copies for v and the output, PE does matmuls + transposes.
"""

import numpy as np

B, T, C, H = 4, 1024, 2048, 32
HD = C // H          # 64
NCORES = 8
HPC = H // NCORES    # 4 heads per core
SC = HPC * HD        # 256 shard channels
NTOK = B * T         # 4096
KT16 = C // 128      # 16 k tiles
MT = NTOK // 128     # 32 token tiles
MPB = T // 128       # 8 token tiles per batch
ROT = 32
NEG = -1.0e9

_CACHE = {}


def _build_nc():
    import concourse.bass as bass
    import concourse.mybir as mybir
    import concourse.tile as tile
    from concourse import bacc
    from concourse.masks import make_identity

    f32 = mybir.dt.float32
    f32r = mybir.dt.float32r
    bf16 = mybir.dt.bfloat16

    nc = bacc.Bacc("TRN2")

    xt_d = nc.dram_tensor("xt", [128, MT, KT16 * 128], bf16, kind="ExternalInput")
    wq_d = nc.dram_tensor("wq", [128, KT16, 3 * SC], bf16, kind="ExternalInput")
    br_d = nc.dram_tensor("br", [128, 2 * SC], f32, kind="ExternalInput")
    c1_d = nc.dram_tensor("c1", [128, MPB, SC], f32, kind="ExternalInput")
    c2_d = nc.dram_tensor("c2", [128, MPB, SC], f32, kind="ExternalInput")
    mk_d = nc.dram_tensor("mk", [128, 1024], bf16, kind="ExternalInput")
    w2_d = nc.dram_tensor("w2", [128, 2, C], bf16, kind="ExternalInput")
    out_d = nc.dram_tensor("out", [MT, 128, C], bf16, kind="ExternalOutput")

    with tile.TileContext(nc) as tc:
        with (
            tc.tile_pool(name="const", bufs=1) as const,
            tc.tile_pool(name="xp", bufs=3) as xp,
            tc.tile_pool(name="qkvp", bufs=3) as qkvp,
            tc.tile_pool(name="rtp", bufs=2) as rtp,
            tc.tile_pool(name="bigp", bufs=2) as bigp,
            tc.tile_pool(name="ptp", bufs=12) as ptp,
            tc.tile_pool(name="outp", bufs=3) as outp,
            tc.tile_pool(name="rsp", bufs=2) as rsp,
            tc.tile_pool(name="ps", bufs=5, space="PSUM") as ps,
            tc.tile_pool(name="tp", bufs=1, space="PSUM") as tpps,
            tc.tile_pool(name="pc", bufs=2, space="PSUM") as pc,
        ):
            ident = const.tile([128, 128], bf16)
            make_identity(nc, ident)
            ones64 = const.tile([1, 64], bf16)
            nc.vector.memset(ones64[:], 1.0)
            wq = const.tile([128, KT16, 3 * SC], bf16)
            nc.sync.dma_start(wq[:], wq_d[:])
            w2 = const.tile([128, 2, C], bf16)
            nc.sync.dma_start(w2[:], w2_d[:])
            br = const.tile([128, 2 * SC], f32)
            nc.sync.dma_start(br[:], br_d[:])
            c1 = const.tile([128, MPB, SC], f32)
            nc.sync.dma_start(c1[:], c1_d[:])
            c2 = const.tile([128, MPB, SC], f32)
            nc.sync.dma_start(c2[:], c2_d[:])
            mk = const.tile([128, 1024], bf16)
            nc.sync.dma_start(mk[:], mk_d[:])

            for b in range(B):
                QT = bigp.tile([128, 2, T], bf16, tag="qt")
                KTt = bigp.tile([128, 2, T], bf16, tag="kt")
                Vp = bigp.tile([128, MPB, HPC, HD + 1], bf16, tag="vp")
                ctxT = bigp.tile([128, 2, T], bf16, tag="ct")
                nc.vector.memset(Vp[:, :, :, HD:HD + 1], 1.0)

                # ---- phase 1: qkv + rope + transpose ----
                for m8 in range(MPB):
                    m = b * MPB + m8
                    xt = xp.tile([128, KT16, 128], bf16)
                    nc.sync.dma_start(
                        xt[:], xt_d[:, m, :].rearrange("p (k t) -> p k t", k=KT16))
                    psA = ps.tile([128, 512], f32, tag="ps")
                    psB = ps.tile([128, 512], f32, tag="ps")
                    for k in range(KT16):
                        nc.tensor.matmul(
                            psA[:], xt[:, k, :], wq[:, k, 0:512],
                            start=(k == 0), stop=(k == KT16 - 1))
                    for k in range(KT16):
                        nc.tensor.matmul(
                            psB[:, 0:256], xt[:, k, :], wq[:, k, 512:768],
                            start=(k == 0), stop=(k == KT16 - 1))
                    # q/k: bias add + rope -> bf16 staging
                    qkv = qkvp.tile([128, 512], f32)
                    nc.vector.tensor_add(qkv[:], psA[:], br[:])
                    qk16 = qkvp.tile([128, 512], bf16, tag="qk16")
                    c1v = c1[:, m8, :].rearrange("p (h d) -> p h d", h=HPC)
                    c2v = c2[:, m8, :].rearrange("p (h d) -> p h d", h=HPC)
                    for base in (0, 256):
                        sec = qkv[:, base:base + 256].rearrange(
                            "p (h d) -> p h d", h=HPC)
                        dst = qk16[:, base:base + 256].rearrange(
                            "p (h d) -> p h d", h=HPC)
                        rt = rtp.tile([128, 256], f32)
                        rtv = rt.rearrange("p (h d) -> p h d", h=HPC)
                        nc.vector.tensor_mul(
                            rtv[:, :, 0:16], sec[:, :, 16:32], c2v[:, :, 0:16])
                        nc.vector.tensor_mul(
                            rtv[:, :, 16:32], sec[:, :, 0:16], c2v[:, :, 16:32])
                        nc.vector.tensor_mul(dst[:], sec[:], c1v)
                        nc.vector.tensor_add(
                            dst[:, :, 0:ROT], dst[:, :, 0:ROT], rtv[:, :, 0:ROT])
                    # v: straight copy into Vp (token-major; bias folded on
                    # host) -- after rope so the PE transposes aren't stalled
                    nc.vector.tensor_copy(
                        Vp[:, m8, :, 0:HD],
                        psB[:, 0:256].rearrange("p (h d) -> p h d", h=HPC))
                    # transpose q/k -> QT/KT (bf16, 1 cyc/row)
                    tp = tpps.tile([128, 1024], bf16)
                    for si, (base, dstT) in enumerate(((0, QT), (256, KTt))):
                        for ci in range(2):
                            nc.tensor.transpose(
                                tp[:, si * 512 + ci * 128:
                                   si * 512 + (ci + 1) * 128],
                                qk16[:, base + ci * 128: base + (ci + 1) * 128],
                                ident)
                        nc.scalar.copy(
                            dstT[:, :, m8 * 128:(m8 + 1) * 128],
                            tp[:, si * 512: si * 512 + 256]
                            .rearrange("p (c t) -> p c t", c=2))

                # ---- phase 2: attention ----
                # Two qb-groups per head are interleaved on the PE so the
                # exp/mask round-trip of one group hides behind the other's
                # matmuls (keeps the PE dense -> full p-state clock).
                # Normalize tails are likewise deferred into the next head's
                # score stream.
                pending_tails = []

                def emit_score(g):
                    st = g["scored"]
                    # exact-causal: a diagonal tile (r >= 0) only needs the
                    # q-range at/above its s-range -> narrow the window
                    r = st - 4 * g["qb"]
                    qoff = 128 * r if r >= 0 else 0
                    w = 512 - qoff
                    stp = ps.tile([128, 512], f32, tag="ps")
                    nc.tensor.matmul(
                        stp[:, 0:w], g["kt"][:, st * 128:(st + 1) * 128],
                        g["qt"][:, g["qb"] * 512 + qoff:(g["qb"] + 1) * 512],
                        start=True, stop=True)
                    pt = ptp.tile([128, 512], bf16)
                    nc.scalar.activation(
                        pt[:, 0:w], stp[:, 0:w],
                        mybir.ActivationFunctionType.Exp)
                    # causal zeroing of the 128-wide diagonal block (0/1 bf16
                    # mask, post-exp, Pool engine: sbuf-only)
                    if r >= 0:
                        nc.gpsimd.tensor_mul(
                            pt[:, 0:128], pt[:, 0:128], mk[:, 512:640])
                    g["pts"].append((pt, qoff, w))
                    g["scored"] += 1

                def emit_av(g):
                    i = g["av"]
                    pt, qoff, w = g["pts"][i]
                    nc.tensor.matmul(
                        g["pct"][0:HD + 1, qoff:512], Vp[:, i, g["h"], :],
                        pt[:, 0:w],
                        start=(i == 0), stop=(i == g["nst"] - 1))
                    g["av"] += 1

                def emit_tail_pre(g):
                    # denom row -> sbuf in bf16 (ACT honors the partition
                    # offset; the custom DVE reciprocal does not)
                    rs_s = rsp.tile([1, 512], bf16, tag="rss")
                    nc.scalar.copy(rs_s[:], g["pct"][HD:HD + 1, :])
                    g["rs_s"] = rs_s

                def make_tail(g):
                    def tail():
                        # broadcast denom across 64 partitions via a rank-1
                        # bf16 PE matmul into the spare upper half of the
                        # pct bank (avoids GpSimd ucode-library thrash of
                        # partition_broadcast), then reciprocal + ctx write
                        nc.tensor.matmul(
                            g["pct"][64:128, :], ones64[:], g["rs_s"][:],
                            start=True, stop=True, skip_group_check=True)
                        den = rsp.tile([HD, 512], f32, tag="den")
                        nc.vector.tensor_copy(den[:], g["pct"][64:128, :])
                        rsb = rsp.tile([HD, 512], f32, tag="rsb")
                        nc.vector.reciprocal_approx_fast(rsb[:], den[:])
                        nc.vector.tensor_mul(
                            ctxT[g["p0"]:g["p0"] + 64, g["h"] // 2,
                                 g["qb"] * 512:(g["qb"] + 1) * 512],
                            g["pct"][0:HD, :], rsb[:])
                    return tail

                LA = 2
                for h in range(HPC):
                    p0 = (h % 2) * 64
                    qt_h = QT[p0:p0 + 64, h // 2, :]
                    kt_h = KTt[p0:p0 + 64, h // 2, :]
                    groups = []
                    for qb in range(2):
                        pct = pc.tile([128, 512], f32, tag="pc")
                        groups.append(dict(
                            qb=qb, h=h, p0=p0, qt=qt_h, kt=kt_h, pct=pct,
                            nst=4 * (qb + 1), pts=[], scored=0, av=0))
                    emitted = 0
                    while any(g["scored"] < g["nst"] for g in groups):
                        for g in groups:
                            if g["scored"] < g["nst"]:
                                emit_score(g)
                                emitted += 1
                                if emitted in (2, 4) and pending_tails:
                                    pending_tails.pop(0)()
                        for g in groups:
                            if (g["av"] < g["nst"]
                                    and g["av"] <= g["scored"] - 1 - LA):
                                emit_av(g)
                    for g in groups:
                        while g["av"] < g["nst"]:
                            emit_av(g)
                        emit_tail_pre(g)
                        pending_tails.append(make_tail(g))
                while pending_tails:
                    pending_tails.pop(0)()

                # ---- phase 3: out projection partial ----
                for m8 in range(MPB):
                    m = b * MPB + m8
                    ot = outp.tile([128, C], bf16)
                    for n in range(4):
                        po = ps.tile([128, 512], f32, tag="ps")
                        for j in range(2):
                            nc.tensor.matmul(
                                po[:], ctxT[:, j, m8 * 128:(m8 + 1) * 128],
                                w2[:, j, n * 512:(n + 1) * 512],
                                start=(j == 0), stop=(j == 1))
                        if n % 2 == 0:
                            nc.scalar.copy(ot[:, n * 512:(n + 1) * 512], po[:])
                        else:
                            nc.vector.tensor_copy(
                                ot[:, n * 512:(n + 1) * 512], po[:])
                    nc.sync.dma_start(out_d[m, :, :], ot[:])

    nc.finalize()
    return nc


def _host_prep(x, rope, Wqkv_w, Wqkv_b, out_w):
    """Build per-core input maps (partition-first layouts, bf16 matmul ins)."""
    import ml_dtypes
    bf = ml_dtypes.bfloat16

    xf = np.ascontiguousarray(x.reshape(NTOK, C)).astype(np.float32)
    # xt[p, m, k*128 + t] = x[m*128+t, k*128+p]
    xt = np.ascontiguousarray(
        xf.reshape(MT, 128, KT16, 128).transpose(3, 0, 2, 1)
        .reshape(128, MT, KT16 * 128)).astype(bf)

    # rope tables (position within a batch: t = 0..1023)
    cos = rope[:, :, 0].astype(np.float32)   # [T, 16]
    sin = rope[:, :, 1].astype(np.float32)
    C1h = np.ones((T, HD), np.float32)
    C1h[:, 0:16] = cos
    C1h[:, 16:32] = cos
    C2h = np.zeros((T, HD), np.float32)
    C2h[:, 0:16] = -sin
    C2h[:, 16:32] = sin
    C1 = np.tile(C1h, (1, HPC))              # [T, 256]
    C2 = np.tile(C2h, (1, HPC))
    # c1[p, q, j] = C1[q*128+p, j]
    c1 = np.ascontiguousarray(C1.reshape(MPB, 128, SC).transpose(1, 0, 2))
    c2 = np.ascontiguousarray(C2.reshape(MPB, 128, SC).transpose(1, 0, 2))

    # causal keep-mask table: mk[p, y] = 0 if y < p + 512 else 1
    yy = np.arange(1024)[None, :]
    pp = np.arange(128)[:, None]
    mk = np.where(yy < pp + 512, 0.0, 1.0).astype(bf)

    scale = np.float32(1.0 / np.sqrt(HD))
    in_maps = []
    for g in range(NCORES):
        hs = g * SC
        Wq = Wqkv_w[hs:hs + SC, :].astype(np.float32) * scale
        Wk = Wqkv_w[C + hs:C + hs + SC, :].astype(np.float32)
        Wv = Wqkv_w[2 * C + hs:2 * C + hs + SC, :].astype(np.float32)
        Wsh = np.concatenate([Wq, Wk, Wv], axis=0)          # [768, 2048]
        # wq[p, k, j] = Wsh[j, k*128+p]
        wqa = np.ascontiguousarray(
            Wsh.T.reshape(KT16, 128, 3 * SC).transpose(1, 0, 2)).astype(bf)
        bq = Wqkv_b[hs:hs + SC].astype(np.float32) * scale
        bk = Wqkv_b[C + hs:C + hs + SC].astype(np.float32)
        bsh = np.concatenate([bq, bk])
        bra = np.ascontiguousarray(np.broadcast_to(bsh, (128, 2 * SC)))
        # w2[p, j, o] = out_w[o, g*256 + j*128 + p]
        w2a = np.ascontiguousarray(
            out_w[:, hs:hs + SC].astype(np.float32).T.reshape(
                2, 128, C).transpose(1, 0, 2)).astype(bf)
        in_maps.append({
            "xt": xt, "wq": wqa, "br": bra, "c1": c1, "c2": c2,
            "mk": mk, "w2": w2a,
        })
    return in_maps


def kernel(x, mask, index, rope, Wqkv_w, Wqkv_b, out_w, out_b,
           k_cache, v_cache):
    from concourse.bass_utils import run_bass_kernel_spmd

    x = np.asarray(x)
    rope = np.asarray(rope)
    Wqkv_w = np.asarray(Wqkv_w)
    Wqkv_b = np.asarray(Wqkv_b)
    out_w = np.asarray(out_w)
    out_b = np.asarray(out_b)

    if "nc" not in _CACHE:
        _CACHE["nc"] = _build_nc()
    nc = _CACHE["nc"]

    in_maps = _host_prep(x, rope, Wqkv_w, Wqkv_b, out_w)
    res = run_bass_kernel_spmd(nc, in_maps, core_ids=list(range(NCORES)))

    acc = np.zeros((NTOK, C), np.float32)
    for g in range(NCORES):
        acc += res.results[g]["out"].reshape(NTOK, C).astype(np.float32)
    # out bias + v-bias folded through the output projection
    bv = Wqkv_b[2 * C:3 * C].astype(np.float32)
    acc += out_b.astype(np.float32) + bv @ out_w.astype(np.float32).T
    return acc.reshape(B, T, C)


# revision 36
# speedup vs baseline: 2.2409x; 1.0978x over previous
"""Trainium2 Bass kernel for nn_MHA_9603546874182.

Causal MHA: qkv proj + rope(32) + causal attention + out proj.
B=4, T=1024, C=2048, H=32, hd=64.

Sharding: 8-way tensor parallel over heads (4 heads / core).
Each core computes qkv for its 4 heads (column-parallel), rope,
causal attention, and a row-parallel partial of the output
projection. Host sums the 8 bf16 partials (+ bias, incl. the v-bias
contribution folded through out_w).

All matmuls run in bf16 (1 cycle/row on the PE at 2.4 GHz vs the
2-4x slower fp32r path measured on HW). Work is spread across
engines: DVE does rope/bias/normalize, ACT does exp + the q/k
transpose-psum evacuations, Pool (gpsimd) does mask adds + psum->
sbuf copies for v and the output, PE does matmuls + transposes.
"""

import numpy as np

B, T, C, H = 4, 1024, 2048, 32
HD = C // H          # 64
NCORES = 8
HPC = H // NCORES    # 4 heads per core
SC = HPC * HD        # 256 shard channels
NTOK = B * T         # 4096
KT16 = C // 128      # 16 k tiles
MT = NTOK // 128     # 32 token tiles
MPB = T // 128       # 8 token tiles per batch
ROT = 32
NEG = -1.0e9

_CACHE = {}


def _build_nc():
    import concourse.bass as bass
    import concourse.mybir as mybir
    import concourse.tile as tile
    from concourse import bacc
    from concourse.masks import make_identity

    f32 = mybir.dt.float32
    f32r = mybir.dt.float32r
    bf16 = mybir.dt.bfloat16

    nc = bacc.Bacc("TRN2")

    xt_d = nc.dram_tensor("xt", [128, MT, KT16 * 128], bf16, kind="ExternalInput")
    wq_d = nc.dram_tensor("wq", [128, KT16, 3 * SC], bf16, kind="ExternalInput")
    br_d = nc.dram_tensor("br", [128, 2 * SC], f32, kind="ExternalInput")
    c1_d = nc.dram_tensor("c1", [128, MPB, SC], f32, kind="ExternalInput")
    c2_d = nc.dram_tensor("c2", [128, MPB, SC], f32, kind="ExternalInput")
    mk_d = nc.dram_tensor("mk", [128, 1024], bf16, kind="ExternalInput")
    w2_d = nc.dram_tensor("w2", [128, 2, C], bf16, kind="ExternalInput")
    out_d = nc.dram_tensor("out", [MT, 128, C], bf16, kind="ExternalOutput")

    with tile.TileContext(nc) as tc:
        with (
            tc.tile_pool(name="const", bufs=1) as const,
            tc.tile_pool(name="xp", bufs=3) as xp,
            tc.tile_pool(name="qkvp", bufs=3) as qkvp,
            tc.tile_pool(name="rtp", bufs=2) as rtp,
            tc.tile_pool(name="bigp", bufs=2) as bigp,
            tc.tile_pool(name="ptp", bufs=12) as ptp,
            tc.tile_pool(name="outp", bufs=3) as outp,
            tc.tile_pool(name="rsp", bufs=2) as rsp,
            tc.tile_pool(name="ps", bufs=5, space="PSUM") as ps,
            tc.tile_pool(name="tp", bufs=1, space="PSUM") as tpps,
            tc.tile_pool(name="pc", bufs=2, space="PSUM") as pc,
        ):
            ident = const.tile([128, 128], bf16)
            make_identity(nc, ident)
            ones64 = const.tile([1, 64], bf16)
            nc.vector.memset(ones64[:], 1.0)
            wq = const.tile([128, KT16, 3 * SC], bf16)
            nc.sync.dma_start(wq[:], wq_d[:])
            w2 = const.tile([128, 2, C], bf16)
            nc.sync.dma_start(w2[:], w2_d[:])
            br = const.tile([128, 2 * SC], f32)
            nc.sync.dma_start(br[:], br_d[:])
            c1 = const.tile([128, MPB, SC], f32)
            nc.sync.dma_start(c1[:], c1_d[:])
            c2 = const.tile([128, MPB, SC], f32)
            nc.sync.dma_start(c2[:], c2_d[:])
            mk = const.tile([128, 1024], bf16)
            nc.sync.dma_start(mk[:], mk_d[:])

            ph3q = []

            for b in range(B):
                QT = bigp.tile([128, 2, T], bf16, tag="qt")
                KTt = bigp.tile([128, 2, T], bf16, tag="kt")
                Vp = bigp.tile([128, MPB, HPC, HD + 1], bf16, tag="vp")
                ctxT = bigp.tile([128, 2, T], bf16, tag="ct")
                nc.vector.memset(Vp[:, :, :, HD:HD + 1], 1.0)

                # ---- phase 1: qkv + rope + transpose ----
                for m8 in range(MPB):
                    m = b * MPB + m8
                    xt = xp.tile([128, KT16, 128], bf16)
                    nc.sync.dma_start(
                        xt[:], xt_d[:, m, :].rearrange("p (k t) -> p k t", k=KT16))
                    psA = ps.tile([128, 512], f32, tag="ps")
                    psB = ps.tile([128, 512], f32, tag="ps")
                    for k in range(KT16):
                        nc.tensor.matmul(
                            psA[:], xt[:, k, :], wq[:, k, 0:512],
                            start=(k == 0), stop=(k == KT16 - 1))
                    for k in range(KT16):
                        nc.tensor.matmul(
                            psB[:, 0:256], xt[:, k, :], wq[:, k, 512:768],
                            start=(k == 0), stop=(k == KT16 - 1))
                    # q/k: bias add + rope -> bf16 staging
                    qkv = qkvp.tile([128, 512], f32)
                    nc.vector.tensor_add(qkv[:], psA[:], br[:])
                    qk16 = qkvp.tile([128, 512], bf16, tag="qk16")
                    c1v = c1[:, m8, :].rearrange("p (h d) -> p h d", h=HPC)
                    c2v = c2[:, m8, :].rearrange("p (h d) -> p h d", h=HPC)
                    for base in (0, 256):
                        sec = qkv[:, base:base + 256].rearrange(
                            "p (h d) -> p h d", h=HPC)
                        dst = qk16[:, base:base + 256].rearrange(
                            "p (h d) -> p h d", h=HPC)
                        rt = rtp.tile([128, 256], f32)
                        rtv = rt.rearrange("p (h d) -> p h d", h=HPC)
                        nc.vector.tensor_mul(
                            rtv[:, :, 0:16], sec[:, :, 16:32], c2v[:, :, 0:16])
                        nc.vector.tensor_mul(
                            rtv[:, :, 16:32], sec[:, :, 0:16], c2v[:, :, 16:32])
                        nc.vector.tensor_mul(dst[:], sec[:], c1v)
                        nc.vector.tensor_add(
                            dst[:, :, 0:ROT], dst[:, :, 0:ROT], rtv[:, :, 0:ROT])
                    # v: straight copy into Vp (token-major; bias folded on
                    # host) -- after rope so the PE transposes aren't stalled
                    nc.vector.tensor_copy(
                        Vp[:, m8, :, 0:HD],
                        psB[:, 0:256].rearrange("p (h d) -> p h d", h=HPC))
                    # transpose q/k -> QT/KT (bf16, 1 cyc/row)
                    tp = tpps.tile([128, 1024], bf16)
                    for si, (base, dstT) in enumerate(((0, QT), (256, KTt))):
                        for ci in range(2):
                            nc.tensor.transpose(
                                tp[:, si * 512 + ci * 128:
                                   si * 512 + (ci + 1) * 128],
                                qk16[:, base + ci * 128: base + (ci + 1) * 128],
                                ident)
                        nc.scalar.copy(
                            dstT[:, :, m8 * 128:(m8 + 1) * 128],
                            tp[:, si * 512: si * 512 + 256]
                            .rearrange("p (c t) -> p c t", c=2))

                # ---- phase 2: attention ----
                # Two qb-groups per head are interleaved on the PE so the
                # exp/mask round-trip of one group hides behind the other's
                # matmuls (keeps the PE dense -> full p-state clock).
                # Normalize tails are likewise deferred into the next head's
                # score stream.
                pending_tails = []

                def emit_score(g):
                    st = g["scored"]
                    # exact-causal: a diagonal tile (r >= 0) only needs the
                    # q-range at/above its s-range -> narrow the window
                    r = st - 4 * g["qb"]
                    qoff = 128 * r if r >= 0 else 0
                    w = 512 - qoff
                    stp = ps.tile([128, 512], f32, tag="ps")
                    nc.tensor.matmul(
                        stp[:, 0:w], g["kt"][:, st * 128:(st + 1) * 128],
                        g["qt"][:, g["qb"] * 512 + qoff:(g["qb"] + 1) * 512],
                        start=True, stop=True)
                    pt = ptp.tile([128, 512], bf16)
                    nc.scalar.activation(
                        pt[:, 0:w], stp[:, 0:w],
                        mybir.ActivationFunctionType.Exp)
                    # causal zeroing of the 128-wide diagonal block (0/1 bf16
                    # mask, post-exp, Pool engine: sbuf-only)
                    if r >= 0:
                        nc.gpsimd.tensor_mul(
                            pt[:, 0:128], pt[:, 0:128], mk[:, 512:640])
                    g["pts"].append((pt, qoff, w))
                    g["scored"] += 1

                def emit_av(g):
                    i = g["av"]
                    pt, qoff, w = g["pts"][i]
                    nc.tensor.matmul(
                        g["pct"][0:HD + 1, qoff:512], Vp[:, i, g["h"], :],
                        pt[:, 0:w],
                        start=(i == 0), stop=(i == g["nst"] - 1))
                    g["av"] += 1

                def emit_tail_pre(g):
                    # denom row -> sbuf in bf16 (ACT honors the partition
                    # offset; the custom DVE reciprocal does not)
                    rs_s = rsp.tile([1, 512], bf16, tag="rss")
                    nc.scalar.copy(rs_s[:], g["pct"][HD:HD + 1, :])
                    g["rs_s"] = rs_s

                def make_tail(g):
                    def tail():
                        # broadcast denom across 64 partitions via a rank-1
                        # bf16 PE matmul into the spare upper half of the
                        # pct bank (avoids GpSimd ucode-library thrash of
                        # partition_broadcast), then reciprocal + ctx write
                        nc.tensor.matmul(
                            g["pct"][64:128, :], ones64[:], g["rs_s"][:],
                            start=True, stop=True, skip_group_check=True)
                        den = rsp.tile([HD, 512], f32, tag="den")
                        nc.vector.tensor_copy(den[:], g["pct"][64:128, :])
                        rsb = rsp.tile([HD, 512], f32, tag="rsb")
                        nc.vector.reciprocal_approx_fast(rsb[:], den[:])
                        nc.vector.tensor_mul(
                            ctxT[g["p0"]:g["p0"] + 64, g["h"] // 2,
                                 g["qb"] * 512:(g["qb"] + 1) * 512],
                            g["pct"][0:HD, :], rsb[:])
                    return tail

                LA = 2
                for h in range(HPC):
                    p0 = (h % 2) * 64
                    qt_h = QT[p0:p0 + 64, h // 2, :]
                    kt_h = KTt[p0:p0 + 64, h // 2, :]
                    groups = []
                    for qb in range(2):
                        pct = pc.tile([128, 512], f32, tag="pc")
                        groups.append(dict(
                            qb=qb, h=h, p0=p0, qt=qt_h, kt=kt_h, pct=pct,
                            nst=4 * (qb + 1), pts=[], scored=0, av=0))
                    emitted = 0
                    while any(g["scored"] < g["nst"] for g in groups):
                        for g in groups:
                            if g["scored"] < g["nst"]:
                                emit_score(g)
                                emitted += 1
                                if emitted in (2, 4) and pending_tails:
                                    pending_tails.pop(0)()
                                if ph3q:
                                    ph3q.pop(0)()
                        for g in groups:
                            if (g["av"] < g["nst"]
                                    and g["av"] <= g["scored"] - 1 - LA):
                                emit_av(g)
                    for g in groups:
                        while g["av"] < g["nst"]:
                            emit_av(g)
                        emit_tail_pre(g)
                        pending_tails.append(make_tail(g))
                while pending_tails:
                    pending_tails.pop(0)()

                # ---- phase 3: out projection partial (deferred:
                # interleaved into the NEXT batch's attention stream so the
                # PE stays dense and fully clocked) ----
                assert not ph3q
                ots = {}

                def make_blk(m8, m, n, ctxT=ctxT):
                    def blk():
                        if n == 0:
                            ot = outp.tile([128, C], bf16)
                            ots[m8] = ot
                        ot = ots[m8]
                        po = ps.tile([128, 512], f32, tag="ps")
                        for j in range(2):
                            nc.tensor.matmul(
                                po[:], ctxT[:, j, m8 * 128:(m8 + 1) * 128],
                                w2[:, j, n * 512:(n + 1) * 512],
                                start=(j == 0), stop=(j == 1))
                        if n % 2 == 0:
                            nc.scalar.copy(ot[:, n * 512:(n + 1) * 512], po[:])
                        else:
                            nc.vector.tensor_copy(
                                ot[:, n * 512:(n + 1) * 512], po[:])
                        if n == 3:
                            nc.sync.dma_start(out_d[m, :, :], ot[:])
                    return blk

                for m8 in range(MPB):
                    for n in range(4):
                        ph3q.append(make_blk(m8, b * MPB + m8, n))

            while ph3q:
                ph3q.pop(0)()

    nc.finalize()
    return nc


def _host_prep(x, rope, Wqkv_w, Wqkv_b, out_w):
    """Build per-core input maps (partition-first layouts, bf16 matmul ins)."""
    import ml_dtypes
    bf = ml_dtypes.bfloat16

    xf = np.ascontiguousarray(x.reshape(NTOK, C)).astype(np.float32)
    # xt[p, m, k*128 + t] = x[m*128+t, k*128+p]
    xt = np.ascontiguousarray(
        xf.reshape(MT, 128, KT16, 128).transpose(3, 0, 2, 1)
        .reshape(128, MT, KT16 * 128)).astype(bf)

    # rope tables (position within a batch: t = 0..1023)
    cos = rope[:, :, 0].astype(np.float32)   # [T, 16]
    sin = rope[:, :, 1].astype(np.float32)
    C1h = np.ones((T, HD), np.float32)
    C1h[:, 0:16] = cos
    C1h[:, 16:32] = cos
    C2h = np.zeros((T, HD), np.float32)
    C2h[:, 0:16] = -sin
    C2h[:, 16:32] = sin
    C1 = np.tile(C1h, (1, HPC))              # [T, 256]
    C2 = np.tile(C2h, (1, HPC))
    # c1[p, q, j] = C1[q*128+p, j]
    c1 = np.ascontiguousarray(C1.reshape(MPB, 128, SC).transpose(1, 0, 2))
    c2 = np.ascontiguousarray(C2.reshape(MPB, 128, SC).transpose(1, 0, 2))

    # causal keep-mask table: mk[p, y] = 0 if y < p + 512 else 1
    yy = np.arange(1024)[None, :]
    pp = np.arange(128)[:, None]
    mk = np.where(yy < pp + 512, 0.0, 1.0).astype(bf)

    scale = np.float32(1.0 / np.sqrt(HD))
    in_maps = []
    for g in range(NCORES):
        hs = g * SC
        Wq = Wqkv_w[hs:hs + SC, :].astype(np.float32) * scale
        Wk = Wqkv_w[C + hs:C + hs + SC, :].astype(np.float32)
        Wv = Wqkv_w[2 * C + hs:2 * C + hs + SC, :].astype(np.float32)
        Wsh = np.concatenate([Wq, Wk, Wv], axis=0)          # [768, 2048]
        # wq[p, k, j] = Wsh[j, k*128+p]
        wqa = np.ascontiguousarray(
            Wsh.T.reshape(KT16, 128, 3 * SC).transpose(1, 0, 2)).astype(bf)
        bq = Wqkv_b[hs:hs + SC].astype(np.float32) * scale
        bk = Wqkv_b[C + hs:C + hs + SC].astype(np.float32)
        bsh = np.concatenate([bq, bk])
        bra = np.ascontiguousarray(np.broadcast_to(bsh, (128, 2 * SC)))
        # w2[p, j, o] = out_w[o, g*256 + j*128 + p]
        w2a = np.ascontiguousarray(
            out_w[:, hs:hs + SC].astype(np.float32).T.reshape(
                2, 128, C).transpose(1, 0, 2)).astype(bf)
        in_maps.append({
            "xt": xt, "wq": wqa, "br": bra, "c1": c1, "c2": c2,
            "mk": mk, "w2": w2a,
        })
    return in_maps


def kernel(x, mask, index, rope, Wqkv_w, Wqkv_b, out_w, out_b,
           k_cache, v_cache):
    from concourse.bass_utils import run_bass_kernel_spmd

    x = np.asarray(x)
    rope = np.asarray(rope)
    Wqkv_w = np.asarray(Wqkv_w)
    Wqkv_b = np.asarray(Wqkv_b)
    out_w = np.asarray(out_w)
    out_b = np.asarray(out_b)

    if "nc" not in _CACHE:
        _CACHE["nc"] = _build_nc()
    nc = _CACHE["nc"]

    in_maps = _host_prep(x, rope, Wqkv_w, Wqkv_b, out_w)
    res = run_bass_kernel_spmd(nc, in_maps, core_ids=list(range(NCORES)))

    acc = np.zeros((NTOK, C), np.float32)
    for g in range(NCORES):
        acc += res.results[g]["out"].reshape(NTOK, C).astype(np.float32)
    # out bias + v-bias folded through the output projection
    bv = Wqkv_b[2 * C:3 * C].astype(np.float32)
    acc += out_b.astype(np.float32) + bv @ out_w.astype(np.float32).T
    return acc.reshape(B, T, C)
